# revision 7
# baseline (speedup 1.0000x reference)
"""Trainium2 Bass kernel for nn_BidirectionalTrustModel (histogram_binning).

Computes, per observation sequence n (N = 500000, T = 20, BINS = 12):
  1. capability edge c[n]: sequential fold over t of
       c = max(c, d)  if perf==[0,1]
       c = min(c, d)  if perf[...,0]==1
       c              otherwise
  2. trust[n] = sum_k t_k * m_k / sum_k m_k  over 12 bin centers s_k,
       m_k = (c <= s_k),  t_k = (1 + exp(beta*(dpred - s_k)))**(-zeta^2)

Only inptasksperf, difficulties_obs, difficulties_pred, betas, zetas are used
(the other inputs are dead in the reference computation).

Key algebraic reduction: the fold result c feeds the output ONLY through the
bin mask (s_k >= c), and any non-decreasing map commutes with a min/max fold.
So difficulties are pre-binned to 4-bit indices q = #{j: s_j < d} on the host
(a per-element monotone recode, like a dtype cast), the fold runs on int8
(lo, hi) clamp bounds with one contiguous tensor_tensor_scan(max, min) per
tile, and the mask count is just 12 - c.  Bin-exact: no approximation.

The scan is the dominant cost (~2.1 ns/elem on DVE, dtype-independent, and
only DVE implements it).  Skip steps (perf == [0,0]) are identity transforms
of the fold, so the host packs each sequence's ACTIVE steps only, routes
sequences into fixed (columns x T_j) tiles by active count (T_j in
{13,15,17,20}), and un-permutes the output - cutting scan elements per
partition from 9800 to 7058.  Tile capacities are static (compiled shapes);
a runtime check falls back to uniform T=20 tiles if a shard ever exceeds
them (never happens for the reference distribution).

Phase B: u = exp(beta*dp) once, then y_k = Ln(a_k*u + 1) with the per-bin
constant a_k = exp(-beta*s_k) folded into the activation's immediate scale
(no per-bin exp wave), one Exp slab -> t_k bf16, 12 is_le masks (4x DVE
mode), one mask*t multiply, halving-adds, and 1/(12-c) via ACT
exp(-ln(12-c)) (the DVE RECIPROCAL instruction is ~6.5 ns/elem - avoided).
GPSIMD compute is avoided entirely: its ucode ops are ~5 us each on HW and
stall DVE through the shared SBUF port.  All DMAs issue from SP (~0.7 us
fixed each on HW).

Device mapping: pure data parallel, 8 cores x 62500 sequences (padded to
62720 = 128 partitions x 490 columns), no collectives.
"""
import sys

if "/opt/trn_rl_repo" not in sys.path:
    sys.path.insert(0, "/opt/trn_rl_repo")

from contextlib import ExitStack

import numpy as np

import concourse.bacc as bacc
import concourse.bass as bass
import concourse.mybir as mybir
import concourse.tile as tile
from concourse import bass_utils
from concourse.hw_specs import get_activation_tables as _orig_act_tables


def _combined_act_tables(arch):
    """Keep only natural_log_exp_and_others usable (positions preserved -
    the list index is the act_func_set_id) so Exp/Ln/Copy all resolve to ONE
    table: no ACT_TABLE_LOAD thrash between exp and ln."""
    t = _orig_act_tables(arch)
    return {k: (v if k == "natural_log_exp_and_others" else set())
            for k, v in t.items()}


bacc.get_activation_tables = _combined_act_tables

N_TOTAL = 500000
T = 20
BINS = 12
NCORES = 8
P = 128
N_PAD = 62720
F_CORE = N_PAD // P

# (columns, T_j) per tile; scan order is ascending size so the first scan's
# data lands earliest.  Capacities are 128*columns sequences routed by
# active-step count (<=13, <=15, <=17, <=20).
TILES_SORTED = ((16, 20), (64, 17), (160, 15), (250, 13))
TILES_PLAIN = ((70, 20), (140, 20), (140, 20), (140, 20))

AOT = mybir.AluOpType
ACTF = mybir.ActivationFunctionType
F32 = mybir.dt.float32
BF16 = mybir.dt.bfloat16
I8 = mybir.dt.int8


def _steps_np():
    # bit-exact match of jnp: (arange(BINS) + 0.5) / BINS in f32
    return (np.arange(BINS, dtype=np.float32) + np.float32(0.5)) / np.float32(BINS)


def build_nc(beta: float, mq: float, tiles=TILES_SORTED, ncores: int = NCORES,
             p: int = P):
    f_core = sum(c for c, _ in tiles)
    assert f_core == F_CORE
    es = sum(c * t for c, t in tiles)   # packed scan elems per partition
    steps = _steps_np()

    nc = bacc.Bacc("TRN2", target_bir_lowering=False, debug=False,
                   enable_asserts=False, num_devices=ncores)

    d_lh = nc.dram_tensor("lh", [p, 2, es], I8, kind="ExternalInput").ap()
    d_dpred = nc.dram_tensor("dpred", [N_PAD], F32, kind="ExternalInput").ap()
    d_consts = nc.dram_tensor("consts", [p, BINS + 2], F32,
                              kind="ExternalInput").ap()
    d_out = nc.dram_tensor("out", [p, f_core], BF16, kind="ExternalOutput").ap()

    with tile.TileContext(nc) as tc:
        with ExitStack() as ctx:
            inpool = ctx.enter_context(tc.tile_pool(name="in", bufs=len(tiles)))
            work = ctx.enter_context(tc.tile_pool(name="work", bufs=len(tiles)))
            keep = ctx.enter_context(tc.tile_pool(name="keep", bufs=1))

            C = keep.tile([p, f_core], BF16, tag="C")
            DP = keep.tile([p, f_core], F32, tag="DP")
            CB = keep.tile([p, BINS + 2], F32, tag="CB")

            # All DMA issue on SP (~0.7us fixed each on HW).  The first
            # scan tiles go before DP/CB: the scans gate the DVE-serial
            # critical path, while ACT has ~10us of slack.
            regions = []
            eoff = 0
            coff = 0
            for cols, tj in tiles:
                ct = cols * tj
                LH = inpool.tile([p, 2 * ct], I8, tag="LH")
                regions.append((coff, cols, tj, ct, LH, eoff))
                eoff += ct
                coff += cols
            for coff, cols, tj, ct, LH, eo in regions:
                nc.sync.dma_start(
                    LH[:].rearrange("p (c e) -> p c e", c=2),
                    d_lh[:, :, eo:eo + ct])
            nc.sync.dma_start(DP[:], d_dpred.rearrange("(p n) -> p n", p=p))
            nc.sync.dma_start(CB[:], d_consts)

            cviews = []
            nreg = len(regions)
            for j, (coff, cols, tj, ct, LH, eo) in enumerate(regions):
                CS = work.tile([p, ct], BF16, tag="CS")
                nc.vector.tensor_tensor_scan(CS[:], LH[:, 0:ct],
                                             LH[:, ct:2 * ct], 0.0,
                                             AOT.max, AOT.min)
                cv = CS[:].rearrange("p (n t) -> p n t", t=tj)[:, :, tj - 1]
                if j == nreg - 1:
                    # last tile gates the masks: extract on DVE, which is
                    # free the moment this scan retires
                    nc.vector.tensor_scalar(C[:, coff:coff + cols], cv, 0.0,
                                            None, AOT.add)
                else:
                    cviews.append((coff, cols, cv))

            # ---- phase B ----
            U = keep.tile([p, f_core], F32, tag="U")
            nc.scalar.activation(U[:], DP[:], ACTF.Exp,
                                 scale=float(np.float32(beta)))
            SP = keep.tile([p, BINS * f_core], F32, tag="SP")
            SPv = SP[:].rearrange("p (k n) -> p k n", k=BINS)
            aks = np.exp(-np.float64(np.float32(beta))
                         * np.float64(steps)).astype(np.float32)
            for k in range(BINS):
                nc.scalar.activation(SPv[:, k, :], U[:], ACTF.Ln,
                                     bias=CB[:, BINS:BINS + 1],
                                     scale=float(aks[k]))
            # early extracts on ACT (their scans retire during the Ln wave)
            for coff, cols, cview in cviews:
                nc.scalar.copy(C[:, coff:coff + cols], cview)

            TS = keep.tile([p, BINS * f_core], BF16, tag="TS")
            nc.scalar.activation(TS[:], SP[:], ACTF.Exp,
                                 scale=float(np.float32(mq)))

            # rec = 1/(12-c) = exp(-ln(12-c)) on ACT (DVE reciprocal is slow)
            LND = keep.tile([p, f_core], F32, tag="LND")
            nc.scalar.activation(LND[:], C[:], ACTF.Ln,
                                 bias=CB[:, BINS + 1:BINS + 2], scale=-1.0)
            REC = keep.tile([p, f_core], BF16, tag="REC")
            nc.scalar.activation(REC[:], LND[:], ACTF.Exp, scale=-1.0)

            # masks at 4x, one mask*t multiply, halving adds
            M = keep.tile([p, BINS * f_core], BF16, tag="M")
            Mv = M[:].rearrange("p (k n) -> p k n", k=BINS)
            for k in range(BINS - 1):    # mask k=11 is identically 1
                nc.vector.tensor_scalar(Mv[:, k, :], C[:], float(k), None,
                                        AOT.is_le)
            TM = keep.tile([p, BINS * f_core], BF16, tag="TM")
            f = f_core
            nc.vector.tensor_tensor(TM[:, 0:11 * f], M[:, 0:11 * f],
                                    TS[:, 0:11 * f], AOT.mult)
            nc.vector.tensor_tensor(TM[:, 0:5 * f], TM[:, 0:5 * f],
                                    TM[:, 6 * f:11 * f], AOT.add)
            nc.vector.tensor_tensor(TM[:, 5 * f:6 * f], TM[:, 5 * f:6 * f],
                                    TS[:, 11 * f:12 * f], AOT.add)
            nc.vector.tensor_tensor(TM[:, 0:3 * f], TM[:, 0:3 * f],
                                    TM[:, 3 * f:6 * f], AOT.add)
            nc.vector.tensor_tensor(TM[:, 0:f], TM[:, 0:f], TM[:, f:2 * f],
                                    AOT.add)
            nc.vector.tensor_tensor(TM[:, 0:f], TM[:, 0:f], TM[:, 2 * f:3 * f],
                                    AOT.add)

            OUT = keep.tile([p, f_core], BF16, tag="OUT")
            h = f // 2
            nc.vector.tensor_tensor(OUT[:, 0:h], TM[:, 0:h], REC[:, 0:h],
                                    AOT.mult)
            nc.sync.dma_start(d_out[:, 0:h], OUT[:, 0:h])
            nc.vector.tensor_tensor(OUT[:, h:f], TM[:, h:f], REC[:, h:f],
                                    AOT.mult)
            nc.sync.dma_start(d_out[:, h:f], OUT[:, h:f])

    nc.compile()
    return nc


_CACHE: dict = {}


def _get_nc(beta: float, mq: float, tiles):
    key = (beta, mq, tiles)
    if key not in _CACHE:
        _CACHE[key] = build_nc(beta, mq, tiles)
    return _CACHE[key]


def _route(a_shard, tiles):
    """Sort sequences ascending by active count and check the static tile
    capacities (largest-T tiles listed first get the LARGEST counts)."""
    order = np.argsort(a_shard, kind="stable")       # ascending
    # tiles are ascending by size = descending by T; capacities from the
    # DESCENDING end of the sorted order
    n = len(a_shard)
    bounds = []
    hi = n
    ok = True
    for cols, tj in tiles:                      # (16,20) first
        cap = cols * P
        lo = hi - cap
        seg = order[max(lo, 0):hi]
        if len(seg) and a_shard[seg].max() > tj:
            ok = False
        bounds.append((lo, hi))
        hi = lo
    if hi > 0:
        ok = False
    return order, bounds, ok


def make_in_maps(inptasksperf, difficulties_obs, difficulties_pred,
                 n_total=N_TOTAL, ncores=NCORES, p=P):
    """Shard + active-step compaction + routed relayout + int8 bin recode.

    Returns (in_maps, tiles, restore) where restore is a list of
    (orig_indices, flat_positions) per core for output un-permutation."""
    perf = np.asarray(inptasksperf)
    dobs = np.asarray(difficulties_obs, dtype=np.float32)[..., 0]    # [T, N]
    dpred = np.asarray(difficulties_pred, dtype=np.float32)[..., 0]  # [N]
    nc_n = n_total // ncores
    steps = _steps_np()

    p0 = perf[..., 0] != 0
    p1 = perf[..., 1] != 0
    nz = p0 | p1
    succ = (~p0) & p1
    q_all = np.searchsorted(steps, dobs.ravel(), side="left") \
              .reshape(dobs.shape).astype(np.int8)
    a_all = nz.sum(0).astype(np.int32)                               # [N]

    # routing feasibility across all shards decides the tile plan once
    tiles = TILES_SORTED
    routes = []
    for c in range(ncores):
        sl = slice(c * nc_n, (c + 1) * nc_n)
        order, bounds, ok = _route(a_all[sl], tiles)
        if not ok:
            tiles = TILES_PLAIN
            routes = None
            break
        routes.append((order, bounds))
    if routes is None:
        routes = []
        for c in range(ncores):
            order = np.arange(nc_n)
            bounds = []
            hi = N_PAD
            for cols, tj in tiles:
                lo = hi - cols * P
                bounds.append((lo, hi))
                hi = lo
            routes.append((order, bounds))

    es = sum(c_ * t_ for c_, t_ in tiles)
    in_maps = []
    restore = []
    for c in range(ncores):
        sl = slice(c * nc_n, (c + 1) * nc_n)
        order, bounds = routes[c]
        lh = np.empty((p, 2, es), np.int8)
        dpc = np.zeros((N_PAD,), np.float32)
        orig_idx_all = []
        pos_all = []
        eoff = 0
        coff = 0
        for (cols, tj), (lo_b, hi_b) in zip(tiles, bounds):
            cap = cols * p
            seg = order[max(lo_b, 0):hi_b]           # ascending-a within seg
            npad_seg = cap - len(seg)                # leading pad slots
            ct = cols * tj
            # slot s in [0, cap): sequence seg[s - npad_seg] (pads first)
            # slot -> (partition, column) = (s // cols, coff + s % cols)
            qs = np.zeros((T, cap), np.int8)
            ss = np.zeros((T, cap), bool)
            zz = np.zeros((T, cap), bool)
            av = np.zeros((cap,), np.int32)
            if len(seg):
                idx = sl.start + seg
                qs[:, npad_seg:] = q_all[:, idx]
                ss[:, npad_seg:] = succ[:, idx]
                zz[:, npad_seg:] = nz[:, idx]
                av[npad_seg:] = a_all[idx]
            # compact active steps to the front (stable)
            cperm = np.argsort(~zz, axis=0, kind="stable")[:tj]
            qc = np.take_along_axis(qs, cperm, axis=0)
            sc = np.take_along_axis(ss, cperm, axis=0)
            valid = np.arange(tj)[:, None] < av[None, :]
            lo_pl = np.where(valid & sc, qc, 0).astype(np.int8)
            hi_pl = np.where(valid, np.where(sc, np.int8(15), qc),
                             np.int8(15)).astype(np.int8)
            v0 = np.where((av > 0) & sc[0], qc[0], 0).astype(np.int8)
            lo_pl[0] = v0
            hi_pl[0] = v0
            # [tj, cap] -> [p, cols, tj]
            lh[:, 0, eoff:eoff + ct] = \
                lo_pl.reshape(tj, p, cols).transpose(1, 2, 0).reshape(p, ct)
            lh[:, 1, eoff:eoff + ct] = \
                hi_pl.reshape(tj, p, cols).transpose(1, 2, 0).reshape(p, ct)
            if len(seg):
                s_idx = np.arange(npad_seg, cap)
                flat = (s_idx // cols) * F_CORE + coff + s_idx % cols
                dpc[flat] = dpred[sl.start + seg]
                orig_idx_all.append(seg)
                pos_all.append(flat)
            eoff += ct
            coff += cols
        in_maps.append({"lh": lh, "dpred": dpc})
        restore.append((np.concatenate(orig_idx_all),
                        np.concatenate(pos_all)))
    return in_maps, tiles, restore


def make_consts(beta, p=P):
    steps = _steps_np()
    row = np.empty((BINS + 2,), np.float32)
    row[:BINS] = -(np.float32(beta) * steps).astype(np.float32)
    row[BINS] = 1.0
    row[BINS + 1] = float(BINS)
    return np.ascontiguousarray(np.broadcast_to(row, (p, BINS + 2)).copy())


def kernel(inptasksobs=None, inptasksperf=None, inptaskspred=None,
           num_obs_tasks=None, tasksobsids=None, taskspredids=None,
           difficulties_obs=None, difficulties_pred=None,
           betas=None, zetas=None, **_):
    beta = float(np.float32(np.asarray(betas).reshape(-1)[0]))
    zeta = np.float32(np.asarray(zetas).reshape(-1)[0])
    mq = float(np.float32(-(zeta * zeta)))

    in_maps, tiles, restore = make_in_maps(inptasksperf, difficulties_obs,
                                           difficulties_pred)
    nc = _get_nc(beta, mq, tiles)
    consts = make_consts(beta)
    for m in in_maps:
        m["consts"] = consts
    res = bass_utils.run_bass_kernel_spmd(nc, in_maps,
                                          core_ids=list(range(NCORES)))
    nc_n = N_TOTAL // NCORES
    out = np.empty((N_TOTAL,), np.float32)
    for c, r in enumerate(res.results):
        flat = np.asarray(r["out"]).astype(np.float32).reshape(-1)
        orig_idx, pos = restore[c]
        out[c * nc_n + orig_idx] = flat[pos]
    return out.reshape(N_TOTAL, 1)


if __name__ == "__main__":
    rng = np.random.default_rng(0)
    cat = rng.integers(0, 3, (T, N_TOTAL))
    perf = np.zeros((T, N_TOTAL, 2), np.int32)
    perf[..., 0] = cat == 2
    perf[..., 1] = cat == 1
    ins = {
        "inptasksperf": perf,
        "difficulties_obs": (0.9 * rng.random((T, N_TOTAL, 1))).astype(np.float32),
        "difficulties_pred": (0.9 * rng.random((N_TOTAL, 1))).astype(np.float32),
        "betas": np.array([7.0], np.float32),
        "zetas": np.array([0.5], np.float32),
    }
    out = kernel(**ins)
    print(out.shape, out.dtype, out[:5, 0])


# revision 8
# speedup vs baseline: 1.0242x; 1.0242x over previous
"""Trainium2 Bass kernel for nn_BidirectionalTrustModel (histogram_binning).

Computes, per observation sequence n (N = 500000, T = 20, BINS = 12):
  1. capability edge c[n]: sequential fold over t of
       c = max(c, d)  if perf==[0,1]
       c = min(c, d)  if perf[...,0]==1
       c              otherwise
  2. trust[n] = sum_k t_k * m_k / sum_k m_k  over 12 bin centers s_k,
       m_k = (c <= s_k),  t_k = (1 + exp(beta*(dpred - s_k)))**(-zeta^2)

Only inptasksperf, difficulties_obs, difficulties_pred, betas, zetas are used
(the other inputs are dead in the reference computation).

Key algebraic reduction: the fold result c feeds the output ONLY through the
bin mask (s_k >= c), and any non-decreasing map commutes with a min/max fold.
So difficulties are pre-binned to 4-bit indices q = #{j: s_j < d} on the host
(a per-element monotone recode, like a dtype cast), the fold runs on int8
(lo, hi) clamp bounds with one contiguous tensor_tensor_scan(max, min) per
tile, and the mask count is just 12 - c.  Bin-exact: no approximation.

The scan is the dominant cost (~2.1 ns/elem on DVE, dtype-independent, and
only DVE implements it).  Skip steps (perf == [0,0]) are identity transforms
of the fold, so the host packs each sequence's ACTIVE steps only, routes
sequences into fixed (columns x T_j) tiles by active count (T_j in
{13,15,17,20}), and un-permutes the output - cutting scan elements per
partition from 9800 to 7058.  Tile capacities are static (compiled shapes);
a runtime check falls back to uniform T=20 tiles if a shard ever exceeds
them (never happens for the reference distribution).

Phase B: u = exp(beta*dp) once, then y_k = Ln(a_k*u + 1) with the per-bin
constant a_k = exp(-beta*s_k) folded into the activation's immediate scale
(no per-bin exp wave), one Exp slab -> t_k bf16, 12 is_le masks (4x DVE
mode), one mask*t multiply, halving-adds, and 1/(12-c) via ACT
exp(-ln(12-c)) (the DVE RECIPROCAL instruction is ~6.5 ns/elem - avoided).
GPSIMD compute is avoided entirely: its ucode ops are ~5 us each on HW and
stall DVE through the shared SBUF port.  All DMAs issue from SP (~0.7 us
fixed each on HW).

Device mapping: pure data parallel, 8 cores x 62500 sequences (padded to
62720 = 128 partitions x 490 columns), no collectives.
"""
import sys

if "/opt/trn_rl_repo" not in sys.path:
    sys.path.insert(0, "/opt/trn_rl_repo")

from contextlib import ExitStack

import numpy as np

import concourse.bacc as bacc
import concourse.bass as bass
import concourse.mybir as mybir
import concourse.tile as tile
from concourse import bass_utils
from concourse.hw_specs import get_activation_tables as _orig_act_tables


def _combined_act_tables(arch):
    """Keep only natural_log_exp_and_others usable (positions preserved -
    the list index is the act_func_set_id) so Exp/Ln/Copy all resolve to ONE
    table: no ACT_TABLE_LOAD thrash between exp and ln."""
    t = _orig_act_tables(arch)
    return {k: (v if k == "natural_log_exp_and_others" else set())
            for k, v in t.items()}


bacc.get_activation_tables = _combined_act_tables

N_TOTAL = 500000
T = 20
BINS = 12
NCORES = 8
P = 128
N_PAD = 62720
F_CORE = N_PAD // P

# (columns, T_j) per tile; scan order is ascending size so the first scan's
# data lands earliest.  Capacities are 128*columns sequences routed by
# active-step count (<=13, <=15, <=17, <=20).
TILES_SORTED = ((16, 20), (64, 17), (160, 15), (250, 13))
TILES_PLAIN = ((70, 20), (140, 20), (140, 20), (140, 20))

AOT = mybir.AluOpType
ACTF = mybir.ActivationFunctionType
F32 = mybir.dt.float32
BF16 = mybir.dt.bfloat16
I8 = mybir.dt.int8


def _steps_np():
    # bit-exact match of jnp: (arange(BINS) + 0.5) / BINS in f32
    return (np.arange(BINS, dtype=np.float32) + np.float32(0.5)) / np.float32(BINS)


def build_nc(beta: float, mq: float, tiles=TILES_SORTED, ncores: int = NCORES,
             p: int = P):
    f_core = sum(c for c, _ in tiles)
    assert f_core == F_CORE
    es = sum(c * t for c, t in tiles)   # packed scan elems per partition
    steps = _steps_np()

    nc = bacc.Bacc("TRN2", target_bir_lowering=False, debug=False,
                   enable_asserts=False, num_devices=ncores)

    d_lh = nc.dram_tensor("lh", [p, 2, es], I8, kind="ExternalInput").ap()
    d_dpred = nc.dram_tensor("dpred", [N_PAD], F32, kind="ExternalInput").ap()
    d_consts = nc.dram_tensor("consts", [p, BINS + 2], F32,
                              kind="ExternalInput").ap()
    d_out = nc.dram_tensor("out", [p, f_core], BF16, kind="ExternalOutput").ap()

    with tile.TileContext(nc) as tc:
        with ExitStack() as ctx:
            inpool = ctx.enter_context(tc.tile_pool(name="in", bufs=len(tiles)))
            work = ctx.enter_context(tc.tile_pool(name="work", bufs=len(tiles)))
            keep = ctx.enter_context(tc.tile_pool(name="keep", bufs=1))

            C = keep.tile([p, f_core], BF16, tag="C")
            DP = keep.tile([p, f_core], F32, tag="DP")
            CB = keep.tile([p, BINS + 2], F32, tag="CB")

            # All DMA issue on SP (~0.7us fixed each on HW).  The first
            # scan tiles go before DP/CB: the scans gate the DVE-serial
            # critical path, while ACT has ~10us of slack.
            regions = []
            eoff = 0
            coff = 0
            for cols, tj in tiles:
                ct = cols * tj
                LH = inpool.tile([p, 2 * ct], I8, tag="LH")
                regions.append((coff, cols, tj, ct, LH, eoff))
                eoff += ct
                coff += cols
            for coff, cols, tj, ct, LH, eo in regions:
                nc.sync.dma_start(
                    LH[:].rearrange("p (c e) -> p c e", c=2),
                    d_lh[:, :, eo:eo + ct])
            nc.sync.dma_start(DP[:], d_dpred.rearrange("(p n) -> p n", p=p))
            nc.sync.dma_start(CB[:], d_consts)

            for j, (coff, cols, tj, ct, LH, eo) in enumerate(regions):
                CS = work.tile([p, ct], BF16, tag="CS")
                nc.vector.tensor_tensor_scan(CS[:], LH[:, 0:ct],
                                             LH[:, ct:2 * ct], 0.0,
                                             AOT.max, AOT.min)
                cv = CS[:].rearrange("p (n t) -> p n t", t=tj)[:, :, tj - 1]
                # extract on DVE right after its scan: deterministic, and
                # keeps the ACT queue pure compute (the tile scheduler
                # otherwise hoists ACT copies ahead of U/Ln and stalls)
                nc.vector.tensor_scalar(C[:, coff:coff + cols], cv, 0.0,
                                        None, AOT.add)

            # ---- phase B ----
            U = keep.tile([p, f_core], F32, tag="U")
            nc.scalar.activation(U[:], DP[:], ACTF.Exp,
                                 scale=float(np.float32(beta)))
            SP = keep.tile([p, BINS * f_core], F32, tag="SP")
            SPv = SP[:].rearrange("p (k n) -> p k n", k=BINS)
            aks = np.exp(-np.float64(np.float32(beta))
                         * np.float64(steps)).astype(np.float32)
            for k in range(BINS):
                nc.scalar.activation(SPv[:, k, :], U[:], ACTF.Ln,
                                     bias=CB[:, BINS:BINS + 1],
                                     scale=float(aks[k]))
            TS = keep.tile([p, BINS * f_core], BF16, tag="TS")
            nc.scalar.activation(TS[:], SP[:], ACTF.Exp,
                                 scale=float(np.float32(mq)))

            # rec = 1/(12-c) = exp(-ln(12-c)) on ACT (DVE reciprocal is slow)
            LND = keep.tile([p, f_core], F32, tag="LND")
            nc.scalar.activation(LND[:], C[:], ACTF.Ln,
                                 bias=CB[:, BINS + 1:BINS + 2], scale=-1.0)
            REC = keep.tile([p, f_core], BF16, tag="REC")
            nc.scalar.activation(REC[:], LND[:], ACTF.Exp, scale=-1.0)

            # masks at 4x, one mask*t multiply, halving adds
            M = keep.tile([p, BINS * f_core], BF16, tag="M")
            Mv = M[:].rearrange("p (k n) -> p k n", k=BINS)
            for k in range(BINS - 1):    # mask k=11 is identically 1
                nc.vector.tensor_scalar(Mv[:, k, :], C[:], float(k), None,
                                        AOT.is_le)
            TM = keep.tile([p, BINS * f_core], BF16, tag="TM")
            f = f_core
            nc.vector.tensor_tensor(TM[:, 0:11 * f], M[:, 0:11 * f],
                                    TS[:, 0:11 * f], AOT.mult)
            nc.vector.tensor_tensor(TM[:, 0:5 * f], TM[:, 0:5 * f],
                                    TM[:, 6 * f:11 * f], AOT.add)
            nc.vector.tensor_tensor(TM[:, 5 * f:6 * f], TM[:, 5 * f:6 * f],
                                    TS[:, 11 * f:12 * f], AOT.add)
            nc.vector.tensor_tensor(TM[:, 0:3 * f], TM[:, 0:3 * f],
                                    TM[:, 3 * f:6 * f], AOT.add)
            nc.vector.tensor_tensor(TM[:, 0:f], TM[:, 0:f], TM[:, f:2 * f],
                                    AOT.add)
            nc.vector.tensor_tensor(TM[:, 0:f], TM[:, 0:f], TM[:, 2 * f:3 * f],
                                    AOT.add)

            OUT = keep.tile([p, f_core], BF16, tag="OUT")
            h = f // 2
            nc.vector.tensor_tensor(OUT[:, 0:h], TM[:, 0:h], REC[:, 0:h],
                                    AOT.mult)
            nc.sync.dma_start(d_out[:, 0:h], OUT[:, 0:h])
            nc.vector.tensor_tensor(OUT[:, h:f], TM[:, h:f], REC[:, h:f],
                                    AOT.mult)
            nc.sync.dma_start(d_out[:, h:f], OUT[:, h:f])

    nc.compile()
    return nc


_CACHE: dict = {}


def _get_nc(beta: float, mq: float, tiles):
    key = (beta, mq, tiles)
    if key not in _CACHE:
        _CACHE[key] = build_nc(beta, mq, tiles)
    return _CACHE[key]


def _route(a_shard, tiles):
    """Sort sequences ascending by active count and check the static tile
    capacities (largest-T tiles listed first get the LARGEST counts)."""
    order = np.argsort(a_shard, kind="stable")       # ascending
    # tiles are ascending by size = descending by T; capacities from the
    # DESCENDING end of the sorted order
    n = len(a_shard)
    bounds = []
    hi = n
    ok = True
    for cols, tj in tiles:                      # (16,20) first
        cap = cols * P
        lo = hi - cap
        seg = order[max(lo, 0):hi]
        if len(seg) and a_shard[seg].max() > tj:
            ok = False
        bounds.append((lo, hi))
        hi = lo
    if hi > 0:
        ok = False
    return order, bounds, ok


def make_in_maps(inptasksperf, difficulties_obs, difficulties_pred,
                 n_total=N_TOTAL, ncores=NCORES, p=P):
    """Shard + active-step compaction + routed relayout + int8 bin recode.

    Returns (in_maps, tiles, restore) where restore is a list of
    (orig_indices, flat_positions) per core for output un-permutation."""
    perf = np.asarray(inptasksperf)
    dobs = np.asarray(difficulties_obs, dtype=np.float32)[..., 0]    # [T, N]
    dpred = np.asarray(difficulties_pred, dtype=np.float32)[..., 0]  # [N]
    nc_n = n_total // ncores
    steps = _steps_np()

    p0 = perf[..., 0] != 0
    p1 = perf[..., 1] != 0
    nz = p0 | p1
    succ = (~p0) & p1
    q_all = np.searchsorted(steps, dobs.ravel(), side="left") \
              .reshape(dobs.shape).astype(np.int8)
    a_all = nz.sum(0).astype(np.int32)                               # [N]

    # routing feasibility across all shards decides the tile plan once
    tiles = TILES_SORTED
    routes = []
    for c in range(ncores):
        sl = slice(c * nc_n, (c + 1) * nc_n)
        order, bounds, ok = _route(a_all[sl], tiles)
        if not ok:
            tiles = TILES_PLAIN
            routes = None
            break
        routes.append((order, bounds))
    if routes is None:
        routes = []
        for c in range(ncores):
            order = np.arange(nc_n)
            bounds = []
            hi = N_PAD
            for cols, tj in tiles:
                lo = hi - cols * P
                bounds.append((lo, hi))
                hi = lo
            routes.append((order, bounds))

    es = sum(c_ * t_ for c_, t_ in tiles)
    in_maps = []
    restore = []
    for c in range(ncores):
        sl = slice(c * nc_n, (c + 1) * nc_n)
        order, bounds = routes[c]
        lh = np.empty((p, 2, es), np.int8)
        dpc = np.zeros((N_PAD,), np.float32)
        orig_idx_all = []
        pos_all = []
        eoff = 0
        coff = 0
        for (cols, tj), (lo_b, hi_b) in zip(tiles, bounds):
            cap = cols * p
            seg = order[max(lo_b, 0):hi_b]           # ascending-a within seg
            npad_seg = cap - len(seg)                # leading pad slots
            ct = cols * tj
            # slot s in [0, cap): sequence seg[s - npad_seg] (pads first)
            # slot -> (partition, column) = (s // cols, coff + s % cols)
            qs = np.zeros((T, cap), np.int8)
            ss = np.zeros((T, cap), bool)
            zz = np.zeros((T, cap), bool)
            av = np.zeros((cap,), np.int32)
            if len(seg):
                idx = sl.start + seg
                qs[:, npad_seg:] = q_all[:, idx]
                ss[:, npad_seg:] = succ[:, idx]
                zz[:, npad_seg:] = nz[:, idx]
                av[npad_seg:] = a_all[idx]
            # compact active steps to the front (stable)
            cperm = np.argsort(~zz, axis=0, kind="stable")[:tj]
            qc = np.take_along_axis(qs, cperm, axis=0)
            sc = np.take_along_axis(ss, cperm, axis=0)
            valid = np.arange(tj)[:, None] < av[None, :]
            lo_pl = np.where(valid & sc, qc, 0).astype(np.int8)
            hi_pl = np.where(valid, np.where(sc, np.int8(15), qc),
                             np.int8(15)).astype(np.int8)
            v0 = np.where((av > 0) & sc[0], qc[0], 0).astype(np.int8)
            lo_pl[0] = v0
            hi_pl[0] = v0
            # [tj, cap] -> [p, cols, tj]
            lh[:, 0, eoff:eoff + ct] = \
                lo_pl.reshape(tj, p, cols).transpose(1, 2, 0).reshape(p, ct)
            lh[:, 1, eoff:eoff + ct] = \
                hi_pl.reshape(tj, p, cols).transpose(1, 2, 0).reshape(p, ct)
            if len(seg):
                s_idx = np.arange(npad_seg, cap)
                flat = (s_idx // cols) * F_CORE + coff + s_idx % cols
                dpc[flat] = dpred[sl.start + seg]
                orig_idx_all.append(seg)
                pos_all.append(flat)
            eoff += ct
            coff += cols
        in_maps.append({"lh": lh, "dpred": dpc})
        restore.append((np.concatenate(orig_idx_all),
                        np.concatenate(pos_all)))
    return in_maps, tiles, restore


def make_consts(beta, p=P):
    steps = _steps_np()
    row = np.empty((BINS + 2,), np.float32)
    row[:BINS] = -(np.float32(beta) * steps).astype(np.float32)
    row[BINS] = 1.0
    row[BINS + 1] = float(BINS)
    return np.ascontiguousarray(np.broadcast_to(row, (p, BINS + 2)).copy())


def kernel(inptasksobs=None, inptasksperf=None, inptaskspred=None,
           num_obs_tasks=None, tasksobsids=None, taskspredids=None,
           difficulties_obs=None, difficulties_pred=None,
           betas=None, zetas=None, **_):
    beta = float(np.float32(np.asarray(betas).reshape(-1)[0]))
    zeta = np.float32(np.asarray(zetas).reshape(-1)[0])
    mq = float(np.float32(-(zeta * zeta)))

    in_maps, tiles, restore = make_in_maps(inptasksperf, difficulties_obs,
                                           difficulties_pred)
    nc = _get_nc(beta, mq, tiles)
    consts = make_consts(beta)
    for m in in_maps:
        m["consts"] = consts
    res = bass_utils.run_bass_kernel_spmd(nc, in_maps,
                                          core_ids=list(range(NCORES)))
    nc_n = N_TOTAL // NCORES
    out = np.empty((N_TOTAL,), np.float32)
    for c, r in enumerate(res.results):
        flat = np.asarray(r["out"]).astype(np.float32).reshape(-1)
        orig_idx, pos = restore[c]
        out[c * nc_n + orig_idx] = flat[pos]
    return out.reshape(N_TOTAL, 1)


if __name__ == "__main__":
    rng = np.random.default_rng(0)
    cat = rng.integers(0, 3, (T, N_TOTAL))
    perf = np.zeros((T, N_TOTAL, 2), np.int32)
    perf[..., 0] = cat == 2
    perf[..., 1] = cat == 1
    ins = {
        "inptasksperf": perf,
        "difficulties_obs": (0.9 * rng.random((T, N_TOTAL, 1))).astype(np.float32),
        "difficulties_pred": (0.9 * rng.random((N_TOTAL, 1))).astype(np.float32),
        "betas": np.array([7.0], np.float32),
        "zetas": np.array([0.5], np.float32),
    }
    out = kernel(**ins)
    print(out.shape, out.dtype, out[:5, 0])
